# revision 1
# baseline (speedup 1.0000x reference)
"""GPT-2 (no-softmax attention) dense transformer on 8 TRN2 NeuronCores.

Sharding: core = (batch b, T-half s); b = core//2, s = core%2.
Each core owns the residual stream for (b, s): x[b, s*1024:(s+1)*1024, :],
kept TRANSPOSED in SBUF as xT [C, T_own] fp32 for the whole kernel.

Per layer:
  - qkv = x @ Wqkv.T + b  (bf16 matmuls; k/v of the own T-half are
    AllGathered between the two cores of a pair so both see full-T k,v)
  - att = (q @ k.T) * 1/8, y = att @ v  (full T x T, no softmax/mask)
  - x += y @ Wproj.T + b

All matmul operands bf16: output = inputs_embeds + corrections of
magnitude ~1e-7 (weights are N(0, 2e-4)), so bf16 compute error is
~1e-9 absolute against an O(1) output; the residual add stays fp32.
"""

import sys

if "/opt/trn_rl_repo" not in sys.path:
    sys.path.insert(0, "/opt/trn_rl_repo")

import numpy as np

N_LAYER = 12
N_EMBD = 1024
T_OWN = 1024
B = 4
D = 64

_CACHE = {}


def build(L, C, T_own):
    import concourse.bacc as bacc
    import concourse.mybir as mybir
    from concourse import tile

    f32 = mybir.dt.float32
    bf16 = mybir.dt.bfloat16

    H = C // D
    NCT = C // 128            # 128-wide c tiles
    NTH = max(1, T_own // 512)  # 512-wide t slices of own T
    TW = min(512, T_own)
    NTT = T_own // 128        # own 128-wide t chunks
    T_full = 2 * T_own
    NKT = T_full // 128       # full-T 128-wide k chunks
    NCH = max(1, C // 512)    # 512-wide c_out slices
    CW = min(512, C)
    groups = [[0, 1], [2, 3], [4, 5], [6, 7]]

    nc = bacc.Bacc("TRN2", target_bir_lowering=False, debug=False, num_devices=8)

    xT_in = nc.dram_tensor("xT", [NCT, 128, T_own], f32, kind="ExternalInput")
    wqk_in = nc.dram_tensor("wqk", [L, 2 * NCT, 128, C], bf16, kind="ExternalInput")
    wv_in = nc.dram_tensor("wv", [L, NCT, 128, C], bf16, kind="ExternalInput")
    wp_in = nc.dram_tensor("wp", [L, NCT, 128, C], bf16, kind="ExternalInput")
    bqk_in = nc.dram_tensor("bqk", [L, 128, 2 * NCT], f32, kind="ExternalInput")
    bv_in = nc.dram_tensor("bv", [L, 1, C], bf16, kind="ExternalInput")
    bp_in = nc.dram_tensor("bp", [L, 128, NCT], f32, kind="ExternalInput")
    out_xT = nc.dram_tensor("out", [NCT, 128, T_own], f32, kind="ExternalOutput")

    with tile.TileContext(nc) as tc:
        with (
            tc.tile_pool(name="persist", bufs=1) as persist,
            tc.tile_pool(name="dram", bufs=1, space="DRAM") as dram,
            tc.tile_pool(name="wqk", bufs=4) as wqk_pool,
            tc.tile_pool(name="wvp", bufs=NCT + 1) as wvp_pool,
            tc.tile_pool(name="bias", bufs=2) as bias_pool,
            tc.tile_pool(name="stage", bufs=4) as stage_pool,
            tc.tile_pool(name="attb", bufs=6) as attb_pool,
            tc.tile_pool(name="ystage", bufs=4) as ystage_pool,
            tc.tile_pool(name="pm", bufs=2, space="PSUM") as pm,
            tc.tile_pool(name="pa", bufs=3, space="PSUM") as pa,
            tc.tile_pool(name="py", bufs=2, space="PSUM") as py,
        ):
            xT = persist.tile([128, NCT, T_own], f32)
            xTb = persist.tile([128, NCT, T_own], bf16)
            qt = persist.tile([128, NCT, T_own], bf16)
            kt = persist.tile([128, NCT, T_full], bf16)
            vt = persist.tile([128, NKT, C], bf16)
            yt = persist.tile([128, NCT, T_own], bf16)
            ones = persist.tile([1, 128], bf16)
            nc.gpsimd.memset(ones[:], 1.0)

            sendk = dram.tile([NCT, 128, T_own], bf16)
            sendv = dram.tile([NTT, 128, C], bf16)
            recvk = dram.tile([2, NCT, 128, T_own], bf16)
            recvv = dram.tile([2, NTT, 128, C], bf16)

            for ci in range(NCT):
                nc.sync.dma_start(xT[:, ci, :], xT_in[ci])

            for l in range(L):
                # ---- bias tiles + x -> bf16 cast
                bqk_t = bias_pool.tile([128, 2 * NCT], f32, tag="bqk")
                nc.sync.dma_start(bqk_t[:], bqk_in[l])
                bv_t = bias_pool.tile([1, C], bf16, tag="bv")
                nc.sync.dma_start(bv_t[:], bv_in[l])
                bp_t = bias_pool.tile([128, NCT], f32, tag="bp")
                nc.sync.dma_start(bp_t[:], bp_in[l])
                for ci in range(NCT):
                    nc.vector.tensor_copy(xTb[:, ci, :], xT[:, ci, :])

                # ---- k tiles (co NCT..2NCT-1), staged + AllGather
                for co in range(NCT, 2 * NCT):
                    w = wqk_pool.tile([128, C], bf16, tag="wqk")
                    nc.sync.dma_start(w[:], wqk_in[l, co])
                    st = stage_pool.tile([128, T_own], bf16, tag="st")
                    for th in range(NTH):
                        ps = pm.tile([128, TW], f32, tag="pm")
                        for ci in range(NCT):
                            nc.tensor.matmul(
                                ps[:],
                                w[:, ci * 128 : (ci + 1) * 128],
                                xTb[:, ci, th * TW : (th + 1) * TW],
                                start=(ci == 0),
                                stop=(ci == NCT - 1),
                            )
                        nc.vector.tensor_scalar_add(
                            st[:, th * TW : (th + 1) * TW], ps[:], bqk_t[:, co : co + 1]
                        )
                    nc.sync.dma_start(sendk[co - NCT], st[:])
                nc.gpsimd.collective_compute(
                    "AllGather",
                    mybir.AluOpType.bypass,
                    replica_groups=groups,
                    ins=[sendk.opt()],
                    outs=[recvk.opt()],
                )

                # ---- v tiles (natural [t, c] layout), staged + AllGather
                wv_t = []
                for ci in range(NCT):
                    wvt = wvp_pool.tile([128, C], bf16, tag="wvp")
                    nc.sync.dma_start(wvt[:], wv_in[l, ci])
                    wv_t.append(wvt)
                for tt in range(NTT):
                    st = stage_pool.tile([128, C], bf16, tag="st")
                    for ch in range(NCH):
                        ps = pm.tile([128, CW], f32, tag="pm")
                        for ci in range(NCT):
                            nc.tensor.matmul(
                                ps[:],
                                xTb[:, ci, tt * 128 : (tt + 1) * 128],
                                wv_t[ci][:, ch * CW : (ch + 1) * CW],
                                start=(ci == 0),
                                stop=False,
                            )
                        nc.tensor.matmul(
                            ps[:],
                            ones[:, 0:128],
                            bv_t[:, ch * CW : (ch + 1) * CW],
                            start=False,
                            stop=True,
                        )
                        nc.vector.tensor_copy(st[:, ch * CW : (ch + 1) * CW], ps[:])
                    nc.sync.dma_start(sendv[tt], st[:])
                nc.gpsimd.collective_compute(
                    "AllGather",
                    mybir.AluOpType.bypass,
                    replica_groups=groups,
                    ins=[sendv.opt()],
                    outs=[recvv.opt()],
                )

                # ---- q tiles (co 0..NCT-1) -> qt with scale 1/8 folded
                for co in range(NCT):
                    w = wqk_pool.tile([128, C], bf16, tag="wqk")
                    nc.sync.dma_start(w[:], wqk_in[l, co])
                    for th in range(NTH):
                        ps = pm.tile([128, TW], f32, tag="pm")
                        for ci in range(NCT):
                            nc.tensor.matmul(
                                ps[:],
                                w[:, ci * 128 : (ci + 1) * 128],
                                xTb[:, ci, th * TW : (th + 1) * TW],
                                start=(ci == 0),
                                stop=(ci == NCT - 1),
                            )
                        nc.vector.tensor_scalar(
                            qt[:, co, th * TW : (th + 1) * TW],
                            ps[:],
                            bqk_t[:, co : co + 1],
                            0.125,
                            op0=mybir.AluOpType.add,
                            op1=mybir.AluOpType.mult,
                        )

                # ---- gather k, v back (both halves; uniform across the pair)
                for s in range(2):
                    for j in range(NCT):
                        nc.sync.dma_start(
                            kt[:, j, s * T_own : (s + 1) * T_own], recvk[s, j]
                        )
                    for tt in range(NTT):
                        nc.sync.dma_start(vt[:, s * NTT + tt, :], recvv[s, tt])

                # ---- attention: attT = kT.T-blocks @ q, y accumulated in PSUM
                for h in range(H):
                    j, ro = h // 2, (h % 2) * 64
                    for qi in range(NTH):
                        yp = py.tile([64, TW], f32, tag="py")
                        for ki in range(NKT):
                            ap_ = pa.tile([128, TW], f32, tag="pa")
                            nc.tensor.matmul(
                                ap_[:],
                                kt[ro : ro + 64, j, ki * 128 : (ki + 1) * 128],
                                qt[ro : ro + 64, j, qi * TW : (qi + 1) * TW],
                                start=True,
                                stop=True,
                            )
                            ab = attb_pool.tile([128, TW], bf16, tag="ab")
                            if ki % 2 == 0:
                                nc.vector.tensor_copy(ab[:], ap_[:])
                            else:
                                nc.scalar.activation(
                                    ab[:], ap_[:], mybir.ActivationFunctionType.Copy
                                )
                            nc.tensor.matmul(
                                yp[:],
                                vt[:, ki, h * 64 : (h + 1) * 64],
                                ab[:],
                                start=(ki == 0),
                                stop=(ki == NKT - 1),
                            )
                        ys = ystage_pool.tile([64, TW], bf16, tag="ys")
                        nc.vector.tensor_copy(ys[:], yp[:])
                        nc.sync.dma_start(
                            yt[ro : ro + 64, j, qi * TW : (qi + 1) * TW], ys[:]
                        )

                # ---- proj + residual add into fp32 xT
                wp_t = []
                for ci in range(NCT):
                    wpt = wvp_pool.tile([128, C], bf16, tag="wvp")
                    nc.sync.dma_start(wpt[:], wp_in[l, ci])
                    wp_t.append(wpt)
                for co in range(NCT):
                    for th in range(NTH):
                        ps = pm.tile([128, TW], f32, tag="pm")
                        for ci in range(NCT):
                            nc.tensor.matmul(
                                ps[:],
                                wp_t[ci][:, co * 128 : (co + 1) * 128],
                                yt[:, ci, th * TW : (th + 1) * TW],
                                start=(ci == 0),
                                stop=(ci == NCT - 1),
                            )
                        nc.vector.tensor_scalar_add(ps[:], ps[:], bp_t[:, co : co + 1])
                        nc.vector.tensor_tensor(
                            xT[:, co, th * TW : (th + 1) * TW],
                            xT[:, co, th * TW : (th + 1) * TW],
                            ps[:],
                            op=mybir.AluOpType.add,
                        )

            for ci in range(NCT):
                nc.sync.dma_start(out_xT[ci], xT[:, ci, :])

    nc.compile()
    return nc


def pack_inputs(inputs_embeds, Wqkv, bqkv, Wproj, bproj, L, C, T_own):
    """Host-side shard + relayout. Returns in_maps for the 8 cores."""
    import ml_dtypes

    bf16 = ml_dtypes.bfloat16
    NCT = C // 128
    C2 = 2 * C

    a = Wqkv[:, :C2, :].reshape(L, 2 * NCT, 128, NCT, 128)
    wqk = np.ascontiguousarray(a.transpose(0, 1, 4, 3, 2)).reshape(
        L, 2 * NCT, 128, C
    ).astype(bf16)
    b_ = Wqkv[:, C2 : 3 * C, :].reshape(L, C, NCT, 128)
    wv = np.ascontiguousarray(b_.transpose(0, 2, 3, 1)).astype(bf16)
    pr = Wproj.reshape(L, NCT, 128, NCT, 128)
    wp = np.ascontiguousarray(pr.transpose(0, 3, 4, 1, 2)).reshape(
        L, NCT, 128, C
    ).astype(bf16)
    bqk = np.ascontiguousarray(
        bqkv[:, :C2].reshape(L, 2 * NCT, 128).transpose(0, 2, 1)
    ).astype(np.float32)
    bv = bqkv[:, C2 : 3 * C].reshape(L, 1, C).astype(bf16)
    bp = np.ascontiguousarray(
        bproj.reshape(L, NCT, 128).transpose(0, 2, 1)
    ).astype(np.float32)

    in_maps = []
    for core in range(8):
        b, s = core // 2, core % 2
        xs = inputs_embeds[b, s * T_own : (s + 1) * T_own, :]  # [T_own, C]
        xT = np.ascontiguousarray(xs.T).reshape(NCT, 128, T_own).astype(np.float32)
        in_maps.append(
            {"xT": xT, "wqk": wqk, "wv": wv, "wp": wp, "bqk": bqk, "bv": bv, "bp": bp}
        )
    return in_maps


def run_model(inputs_embeds, Wqkv, bqkv, Wproj, bproj, L, C, T_own, trace=False,
              tmpdir=None):
    from concourse.bass_utils import run_bass_kernel_spmd

    key = (L, C, T_own)
    if key not in _CACHE:
        _CACHE[key] = build(L, C, T_own)
    nc = _CACHE[key]
    in_maps = pack_inputs(inputs_embeds, Wqkv, bqkv, Wproj, bproj, L, C, T_own)
    res = run_bass_kernel_spmd(
        nc, in_maps, core_ids=list(range(8)), trace=trace, tmpdir=tmpdir
    )
    Bfull, T = inputs_embeds.shape[0], inputs_embeds.shape[1]
    out = np.empty((Bfull, T, C), dtype=np.float32)
    for core in range(8):
        b, s = core // 2, core % 2
        o = res.results[core]["out"].reshape(C, T_own)
        out[b, s * T_own : (s + 1) * T_own, :] = o.T
    return out, res


def kernel(**inputs):
    out, _ = run_model(
        inputs["inputs_embeds"],
        inputs["Wqkv"],
        inputs["bqkv"],
        inputs["Wproj"],
        inputs["bproj"],
        N_LAYER,
        N_EMBD,
        T_OWN,
    )
    return out
